# revision 1
# baseline (speedup 1.0000x reference)
"""Trainium2 Bass kernel for nn_CustomLSTM: B=32, S=512, D=512, H=1024.

Strategy (8 NeuronCores, one chip), v2:
  - Tensor-parallel over hidden units: core c owns H-units [128c, 128c+128)
    and computes all four gates for those units (gate order g, i, f, o).
  - No separate input-projection phase: each step's x-projection (4 gates x
    4 k-chunks, N=32) is computed by the PE into the step's PSUM bank during
    the previous step's broadcast dead time; x is streamed from DRAM in
    512-row blocks (16 steps per block). Biases fold into the ACT
    activations (per-partition bias operand).
  - Per step: 32 recurrent bf16 matmuls accumulate W_h^T @ h_{t-1} into one
    PSUM bank ([128 units, 4 gates x 32 batch]), gate-major so ACT overlaps
    the PE; ACT applies tanh/sigmoid per gate; DVE forms c and h; h (bf16)
    is exchanged between the 8 cores with remote_dma_broadcast.
  - Outputs (h, c) accumulate in 16-step SBUF rings and are written to DRAM
    once per 16 steps so the broadcast flight window stays clear of DMA
    descriptor traffic.
  - Raw bass (no Tile): hand-scheduled engine programs with explicit
    semaphores.

Self-contained: hardcodes all shapes; host side shards/reassembles.
"""
import numpy as np
import ml_dtypes

import concourse.bacc as bacc
import concourse.mybir as mybir
from concourse import bass_utils

F32 = mybir.dt.float32
BF16 = mybir.dt.bfloat16

B, S, D, H = 32, 512, 512, 1024
NCORES = 8
UC = H // NCORES          # units per core = 128

import os
if os.environ.get("KERNEL_SIM_STEPS"):
    S = int(os.environ["KERNEL_SIM_STEPS"])
NB = (S * B + 511) // 512   # x row blocks of 512 rows (16 steps each)
NBLK = (S + 15) // 16       # output blocks of 16 steps
ROWS = S * B

_cache = {}

# gate order everywhere: g(0), i(1), f(2), o(3)


def _build(detect_races=True):
    nc = bacc.Bacc(None, target_bir_lowering=False, num_devices=NCORES,
                   detect_race_conditions=detect_races)

    # ---------------- DRAM I/O ----------------
    xT_d = nc.dram_tensor("xT", [128, 4, ROWS], BF16, kind="ExternalInput")
    w_in_d = nc.dram_tensor("w_in", [128, 4, 512], BF16, kind="ExternalInput")
    w_rec_d = nc.dram_tensor("w_rec", [128, 8, 512], BF16, kind="ExternalInput")
    bias_d = nc.dram_tensor("bias", [128, 4], F32, kind="ExternalInput")
    h_out_d = nc.dram_tensor("h_out", [NBLK, 128, 16, 32], F32, kind="ExternalOutput")
    c_out_d = nc.dram_tensor("c_out", [NBLK, 128, 16, 32], F32, kind="ExternalOutput")

    # ---------------- SBUF ----------------
    xt_buf = nc.alloc_sbuf_tensor("xt_buf", [128, 3, 4, 512], BF16)
    w_in_sb = nc.alloc_sbuf_tensor("w_in_sb", [128, 4, 512], BF16)
    w_rec_sb = nc.alloc_sbuf_tensor("w_rec_sb", [128, 8, 512], BF16)
    bias_sb = nc.alloc_sbuf_tensor("bias_sb", [128, 4], F32)
    gact = nc.alloc_sbuf_tensor("gact", [128, 2, 4, 32], BF16)
    f_act = nc.alloc_sbuf_tensor("f_act", [128, 2, 32], F32)
    tmpu = nc.alloc_sbuf_tensor("tmpu", [128, 32], F32)
    tmpv = nc.alloc_sbuf_tensor("tmpv", [128, 32], F32)
    tc_sb = nc.alloc_sbuf_tensor("tc_sb", [128, 2, 32], BF16)
    c_ring = nc.alloc_sbuf_tensor("c_ring", [128, 16, 32], F32)
    h_ring = nc.alloc_sbuf_tensor("h_ring", [128, 16, 32], F32)
    hT_buf = nc.alloc_sbuf_tensor("hT_buf", [128, 2, 8, 32], BF16)
    h_send = nc.alloc_sbuf_tensor("h_send", [128, 2, 32], BF16)

    # bank p*4+g holds gate g of step parity p (cols 0:32)
    pp = nc.alloc_psum_tensor("pp", [128, 8, 512], F32)

    # ---------------- semaphores ----------------
    arr = [nc.alloc_semaphore(f"arr{j}") for j in range(NCORES)]
    loc_sem = nc.alloc_semaphore("loc")
    prep_sem = nc.alloc_semaphore("prep")
    dma_w = nc.alloc_semaphore("dma_w")
    xt_sem = nc.alloc_semaphore("xt_sem")
    xtf = nc.alloc_semaphore("xtf")          # x block consumed by PE
    pg = nc.alloc_semaphore("pg")            # psum gate done: 4 per step
    acts = nc.alloc_semaphore("acts")        # ACT gate ops done: 4 per step
    tcs = nc.alloc_semaphore("tcs")          # tanh(c) done: 1 per step
    cr = nc.alloc_semaphore("cr")            # c written to ring: 1 per step
    hr = nc.alloc_semaphore("hr")            # h staged to ring: 1 per step
    h_ready = nc.alloc_semaphore("h_ready")  # h_send written: 1 per step
    outc_sem = nc.alloc_semaphore("outc_sem")
    outh_sem = nc.alloc_semaphore("outh_sem")

    w_in = w_in_sb.ap()
    w_rec = w_rec_sb.ap()
    ppa = pp.ap()
    hT = hT_buf.ap()
    ga = gact.ap()

    def xpre_mms(pe, t):
        """x-projection matmuls for step t into bank t%2 (start of group)."""
        blk, r = t // 16, t % 16
        if r == 0:
            pe.wait_ge(xt_sem, 64 * (blk + 1))
        p = t % 2
        last = None
        for g in (0, 2, 1, 3):
            for k in range(4):
                last = nc.tensor.matmul(
                    ppa[:, p * 4 + g, 0:32],
                    w_in[:, k, g * 128:(g + 1) * 128],
                    xt_buf.ap()[:, blk % 3, k, r * 32:(r + 1) * 32],
                    start=(k == 0),
                    stop=(t == 0 and k == 3),
                )
            if t == 0:
                last.then_inc(pg, 1)
        if r == 15:
            last.then_inc(xtf, 1)

    with nc.Block() as block:

        # ================= SP: DMA feeder + batched output writer =========
        @block.sync
        def _(sp):
            sp.dma_start(w_in_sb.ap(), w_in_d.ap()).then_inc(dma_w, 16)
            sp.dma_start(w_rec_sb.ap(), w_rec_d.ap()).then_inc(dma_w, 16)
            sp.dma_start(bias_sb.ap(), bias_d.ap()).then_inc(dma_w, 16)
            for n in range(min(3, NB)):
                for q in range(4):
                    sp.dma_start(
                        xt_buf.ap()[:, n, q, :],
                        xT_d.ap()[:, q, n * 512:(n + 1) * 512],
                    ).then_inc(xt_sem, 16)
            for b in range(NBLK):
                sp.wait_ge(cr, 16 * b + 16)
                sp.dma_start(c_out_d.ap()[b], c_ring.ap()).then_inc(outc_sem, 16)
                sp.wait_ge(hr, 16 * b + 16)
                sp.dma_start(h_out_d.ap()[b], h_ring.ap()).then_inc(outh_sem, 16)
                # xt block b+3 reuses block b's slot (consumed at t=16b+14).
                # The 512KB load occupies the DMA engines ~1.4us, so split it
                # into 4 quarter-loads spread over 4 tail windows (t≡5,7,9,11)
                # to keep it out of the broadcast flights.
                n = b + 3
                if n < NB:
                    sp.wait_ge(xtf, b + 1)
                    for q in range(4):
                        sp.wait_ge(cr, 16 * b + 22 + 2 * q)
                        sp.dma_start(
                            xt_buf.ap()[:, n % 3, q, :],
                            xT_d.ap()[:, q, n * 512:(n + 1) * 512],
                        ).then_inc(xt_sem, 16)

        # ================= PE =================
        @block.tensor
        def _(pe):
            pe.wait_ge(dma_w, 48)
            # settle delay: the first broadcast otherwise fires ~5us after
            # kernel start and races device/engine startup, corrupting the
            # first steps (the old phase-1 gave the baseline ~250us of grace)
            if not os.environ.get("KERNEL_SIM_STEPS"):
                for _ in range(2):
                    pe.nop(cycle_cnt=60000)
            # prologue: x-projections for steps 0 and 1
            xpre_mms(pe, 0)
            if S > 1:
                xpre_mms(pe, 1)
            for t in range(1, S):
                p = t % 2
                # recurrent matmuls, issue order g, f, i, o: f before i
                # so the DVE's v=f*c_prev starts one ACT op earlier
                for g in (0, 2, 1, 3):
                    for k in range(NCORES):
                        if g == 0:
                            pe.wait_ge(arr[k], 2 * t)
                        ins = nc.tensor.matmul(
                            ppa[:, p * 4 + g, 0:32],
                            w_rec[:, k, g * 128:(g + 1) * 128],
                            hT[:, (t - 1) % 2, k, :],
                            start=False,
                            stop=(k == NCORES - 1),
                        )
                    ins.then_inc(pg, 1)
                # x-projection for step t+1 (runs in step t's dead time);
                # bank (t+1)%2 must have been drained by ACT at step t-1
                if t + 1 < S:
                    pe.wait_ge(acts, 4 * (t - 1) + 4)
                    xpre_mms(pe, t + 1)

        # ================= ACT =================
        @block.scalar
        def _(act):
            funcs = {
                0: mybir.ActivationFunctionType.Tanh,     # g
                1: mybir.ActivationFunctionType.Sigmoid,  # i
                2: mybir.ActivationFunctionType.Sigmoid,  # f
                3: mybir.ActivationFunctionType.Sigmoid,  # o
            }
            for t in range(S):
                p = t % 2
                for pos, g in enumerate((0, 2, 1, 3)):
                    act.wait_ge(pg, 4 * t + pos + 1)
                    # f (g==2) stays f32: it multiplies the f32 c_prev on
                    # DVE and mixed-dtype DVE inputs are risky on HW
                    out = f_act.ap()[:, p, :] if g == 2 else ga[:, p, g, :]
                    nc.scalar.activation(
                        out,
                        ppa[:, p * 4 + g, 0:32],
                        funcs[g],
                        bias=bias_sb.ap()[:, g:g + 1],
                    ).then_inc(acts, 1)
                act.wait_ge(cr, t + 1)
                nc.scalar.activation(
                    tc_sb.ap()[:, t % 2, :],
                    c_ring.ap()[:, t % 16, :],
                    mybir.ActivationFunctionType.Tanh,
                ).then_inc(tcs, 1)

        # ================= DVE =================
        @block.vector
        def _(dve):
            dve.memset(c_ring.ap()[:, 15, :], 0.0)
            dve.drain()
            for t in range(S):
                p = t % 2
                if t % 16 == 0 and t >= 16:
                    dve.wait_ge(outc_sem, 16 * (t // 16))
                # stage h_{t-1} = o_{t-1} * tanh(c_{t-1}) to the ring
                if t >= 1:
                    if (t - 1) % 16 == 0 and t - 1 >= 16:
                        dve.wait_ge(outh_sem, 16 * ((t - 1) // 16))
                    dve.wait_ge(tcs, t)
                    nc.vector.tensor_mul(
                        h_ring.ap()[:, (t - 1) % 16, :],
                        ga[:, (t - 1) % 2, 3, :],
                        tc_sb.ap()[:, (t - 1) % 2, :],
                    ).then_inc(hr, 1)
                # v = f * c_prev   (acts: g~ is 1st, sigma-f is 2nd)
                dve.wait_ge(acts, 4 * t + 2)
                nc.vector.tensor_mul(
                    tmpv.ap(), f_act.ap()[:, p, :], c_ring.ap()[:, (t - 1) % 16, :]
                )
                # u = i * g~   (sigma-i is 3rd)
                dve.wait_ge(acts, 4 * t + 3)
                nc.vector.tensor_mul(tmpu.ap(), ga[:, p, 1, :], ga[:, p, 0, :])
                dve.drain()
                # c_t = u + v
                nc.vector.tensor_add(
                    c_ring.ap()[:, t % 16, :], tmpu.ap(), tmpv.ap()
                ).then_inc(cr, 1)
                # h_send = o * tanh(c) (bf16) for the broadcast
                if t < S - 1:
                    if t >= 2:
                        dve.wait_ge(loc_sem, 16 * (t - 1))
                    dve.wait_ge(acts, 4 * t + 4)
                    dve.wait_ge(tcs, t + 1)
                    nc.vector.tensor_mul(
                        h_send.ap()[:, p, :], ga[:, p, 3, :], tc_sb.ap()[:, p, :]
                    ).then_inc(h_ready, 1)
            # epilogue: stage h_{S-1}
            dve.wait_ge(tcs, S)
            nc.vector.tensor_mul(
                h_ring.ap()[:, (S - 1) % 16, :],
                ga[:, (S - 1) % 2, 3, :],
                tc_sb.ap()[:, (S - 1) % 2, :],
            ).then_inc(hr, 1)

        # ================= Pool: remote all-gather =================
        @block.gpsimd
        def _(g):
            g.bir_kernel_barrier_wait([list(range(NCORES))])
            pid_reg = g.to_reg(g.partition_id())
            for kcore in range(NCORES):
                with g.If_eq(pid_reg, kcore):
                    for t in range(S - 1):
                        if t >= 1:
                            # descriptor-carveout reclaim: broadcast t-1's
                            # descs must be consumed before regenerating
                            g.wait_ge(loc_sem, 16 * t)
                        g.remote_dma_broadcast(
                            out_ap=hT[:, t % 2, kcore, :],
                            in_ap=h_send.ap()[:, t % 2, :],
                            remote_sem=arr[kcore],
                            local_sem=loc_sem,
                            rdests=[(0, j) for j in range(NCORES)],
                        ).then_inc(prep_sem, 1)
                        g.wait_ge(prep_sem, t + 1)
                        g.wait_ge(h_ready, t + 1)
                        g.trigger_dma(1)

    nc.finalize()
    return nc


def _prep_inputs(x, W_ii, W_if, W_ig, W_io, W_hi, W_hf, W_hg, W_ho,
                 b_i, b_f, b_g, b_o):
    bf = ml_dtypes.bfloat16
    # xT: [D, S, B] -> [4, 128, S*B] -> [128, 4, S*B]; rows ordered (s, b)
    xT = np.ascontiguousarray(
        x.transpose(2, 1, 0).reshape(4, 128, ROWS).transpose(1, 0, 2)
    ).astype(bf)

    in_maps = []
    for c in range(NCORES):
        U = slice(UC * c, UC * (c + 1))
        # gate order [g, i, f, o]
        w_in_c = np.concatenate(
            [W_ig[:, U], W_ii[:, U], W_if[:, U], W_io[:, U]], axis=1
        )  # [512, 512]
        w_in_c = w_in_c.reshape(4, 128, 512).transpose(1, 0, 2).astype(bf)
        w_rec_c = np.concatenate(
            [W_hg[:, U], W_hi[:, U], W_hf[:, U], W_ho[:, U]], axis=1
        )  # [1024, 512]
        w_rec_c = w_rec_c.reshape(8, 128, 512).transpose(1, 0, 2).astype(bf)
        bias_c = np.stack(
            [b_g[U], b_i[U], b_f[U], b_o[U]], axis=1
        ).astype(np.float32)  # [128, 4]
        in_maps.append({
            "xT": xT,
            "w_in": np.ascontiguousarray(w_in_c),
            "w_rec": np.ascontiguousarray(w_rec_c),
            "bias": np.ascontiguousarray(bias_c),
        })
    return in_maps


def run(inputs, trace=False):
    if "nc" not in _cache:
        _cache["nc"] = _build()
    nc = _cache["nc"]
    in_maps = _prep_inputs(**inputs)
    res = bass_utils.run_bass_kernel_spmd(
        nc, in_maps, core_ids=list(range(NCORES)), trace=trace,
    )
    outputs = np.empty((B, S, H), np.float32)
    cells = np.empty((B, S, H), np.float32)
    for c in range(NCORES):
        U = slice(UC * c, UC * (c + 1))
        h = res.results[c]["h_out"]   # [NBLK, 128, 16, 32] = (blk, u, t, b)
        cc = res.results[c]["c_out"]
        outputs[:, :, U] = h.transpose(3, 0, 2, 1).reshape(B, S, 128)
        cells[:, :, U] = cc.transpose(3, 0, 2, 1).reshape(B, S, 128)
    return (outputs, cells), res


def kernel(**inputs):
    (outputs, cells), _ = run(inputs, trace=False)
    return outputs, cells

